# revision 31
# baseline (speedup 1.0000x reference)
"""Trainium2 Bass kernel for nn_CausalFullAttention_13735305413109.

Causal attention with a data-dependent cumprod decay gate and no softmax.
With no softmax the masked quadratic attention is algebraically a chunked
linear attention:
    out_i = q'_i @ State_{blk(i)} + sum_{j<=i, same blk} (q'_i.k'_j) v_j
    State_t = sum_{j < t*BLK} k'_j (x) v_j
with q' = q*SCALE*a_cum, k' = k/max(a_cum,1e-8), per (batch, head).

Sharding: (batch, head-pair) across 8 cores — core c handles batch c//4 and
heads (2*(c%4), 2*(c%4)+1) over that batch's 2048 tokens. Each core emits a
partial out-projection y_part = O_cat @ [w_out[h0]; w_out[h1]] (the in-matmul
sum over its 2 heads); the host sums 4 partials per batch (+ b_out).

Numerics: all matmuls bf16 (1 cyc/row on the PE); the decay recurrence is a
single fp32 cumprod scan of (1 + e^-z) = 1/a capped at 3e37 in-scan, so
ainv = min(scan, 1e8) matches the reference 1/max(a_cum, 1e-8) exactly and
a_cum = approx_recip(scan). rms-norm scale factorization (no-bias build):
k'' = k * (DIM/ss) * ainv  (s^2 folded into k, DIM into wk on host),
v'' = v (unscaled), q'' = q * acum, and the leftover query-side s_i is
applied to the attention output rows at the osb eviction (y_i *= s_i).
Scalar engine uses only Ln/Exp tables (one Ln + Exps, 2 table loads).
"""
import numpy as np
from contextlib import ExitStack

import ml_dtypes
import concourse.bass as bass
import concourse.bacc as bacc
import concourse.mybir as mybir
import concourse.tile as tile
from concourse.bass_utils import run_bass_kernel_spmd

F32 = mybir.dt.float32
BF16 = mybir.dt.bfloat16
AF = mybir.ActivationFunctionType
ALU = mybir.AluOpType

B = 2
N = 2048                # tokens per batch (per core)
DIM = 512
HEADS = 8
DH = 64
BLK = 128
NBLK = N // BLK         # 16
PANEL = 512
NPAN = N // PANEL       # 4
NCHUNK = DIM // 128     # 4
NGRP = 4                # 0=[k0|k1] 1=[v0|v1] 2=[q0|q1] 3=[z0|z1]
SCALE = DH ** -0.5
LOG_SQRT_DIM = float(np.log(np.sqrt(DIM)))
EPS_INV = 1e-8
RAW_CAP = 3e37          # in-scan clamp of 1/acum (keeps fp32 finite)
SKEW = 2                # attention trails proj/gate by this many panels


def build_nc(with_qkv_bias: bool):
    nc = bacc.Bacc()
    x_d = nc.dram_tensor("xT", [128, NCHUNK, N], BF16, kind="ExternalInput")
    w_d = nc.dram_tensor("wall", [128, NCHUNK, NGRP, 128], BF16,
                         kind="ExternalInput")
    wout_d = nc.dram_tensor("wout", [128, DIM], BF16, kind="ExternalInput")
    nba_d = nc.dram_tensor("nba", [128, 1], F32, kind="ExternalInput")
    mask2_d = nc.dram_tensor("mask2", [128, 256], BF16, kind="ExternalInput")
    ident_d = nc.dram_tensor("ident", [128, 128], BF16, kind="ExternalInput")
    if with_qkv_bias:
        bk_d = nc.dram_tensor("bk", [128, 1], F32, kind="ExternalInput")
        bv_d = nc.dram_tensor("bv", [128, 1], F32, kind="ExternalInput")
        bq_d = nc.dram_tensor("bq", [128, 1], F32, kind="ExternalInput")
    y_d = nc.dram_tensor("ypart", [N, DIM], BF16, kind="ExternalOutput")

    with tile.TileContext(nc) as tc, ExitStack() as ctx:
        per = ctx.enter_context(tc.tile_pool(name="persist", bufs=1))
        xsb = per.tile([128, NCHUNK, N], BF16, tag="xsb")
        w_sb = per.tile([128, NCHUNK, NGRP, 128], BF16, tag="wall")
        wout_sb = per.tile([128, DIM], BF16, tag="wout")
        nba_sb = per.tile([128, 1], F32, tag="nba")
        mask2_sb = per.tile([128, 256], BF16, tag="mask2")
        ident_sb = per.tile([128, 128], BF16, tag="ident")
        ones_sb = per.tile([128, 128], BF16, tag="ones")
        lsd_sb = per.tile([128, 1], F32, tag="lsd")
        cap_sb = per.tile([128, PANEL], F32, tag="cap")
        ssAll = per.tile([128, N], F32, tag="ssAll")
        lntAll = per.tile([128, N], F32, tag="lntAll")
        sRep = per.tile([128, N], BF16, tag="sRep")    # sqrt(DIM)/||x_t||
        sRep2 = per.tile([128, N], BF16, tag="sRep2")  # 1/ss (DIM in wk)
        GK = per.tile([128, N], BF16, tag="gk")        # rows [k_h0 | k_h1]
        GV = per.tile([128, N], BF16, tag="gv")
        GQ = per.tile([128, N], BF16, tag="gq")
        GZ = per.tile([128, N], F32, tag="gz")
        araw = per.tile([128, N], F32, tag="araw")     # capped cumprod(1/a)
        ainvb = per.tile([128, N], BF16, tag="ainvb")  # min(araw, 1e8)
        st = per.tile([128, DH], F32, tag="st")        # state accum (fp32)
        stb = per.tile([128, DH], BF16, tag="stb")     # state, bf16 snapshot

        nc.sync.dma_start(w_sb[:], w_d[:])
        nc.sync.dma_start(wout_sb[:], wout_d[:])
        nc.sync.dma_start(nba_sb[:], nba_d[:])
        nc.scalar.dma_start(mask2_sb[:], mask2_d[:])
        nc.scalar.dma_start(ident_sb[:], ident_d[:])
        if with_qkv_bias:
            bk_sb = per.tile([128, 1], F32, tag="bk")
            bv_sb = per.tile([128, 1], F32, tag="bv")
            bq_sb = per.tile([128, 1], F32, tag="bq")
            nc.scalar.dma_start(bk_sb[:], bk_d[:])
            nc.scalar.dma_start(bv_sb[:], bv_d[:])
            nc.scalar.dma_start(bq_sb[:], bq_d[:])
        nc.gpsimd.memset(ones_sb[:], 1.0)
        nc.gpsimd.memset(lsd_sb[:], LOG_SQRT_DIM)
        nc.gpsimd.memset(cap_sb[:], RAW_CAP)
        # x: per (panel, chunk) DMAs, panel-0 first, alternating hwdge queues
        for p in range(NPAN):
            cols = bass.ts(p, PANEL)
            for c in range(NCHUNK):
                eng = nc.sync if c % 2 == 0 else nc.scalar
                eng.dma_start(xsb[:, c, cols], x_d[:, c, cols])

        with (
            tc.tile_pool(name="x2", bufs=2) as x2p,
            tc.tile_pool(name="gat", bufs=2) as gatp,
            tc.tile_pool(name="vkt", bufs=4) as vktp,
            tc.tile_pool(name="ssb", bufs=4) as ssbp,
            tc.tile_pool(name="osb", bufs=3) as osbp,
            tc.tile_pool(name="ysb", bufs=3) as ysbp,
            tc.tile_pool(name="psBig", bufs=2, space="PSUM") as psBig,
            tc.tile_pool(name="psTR", bufs=2, space="PSUM") as psTR,
            tc.tile_pool(name="psSP", bufs=2, space="PSUM") as psSP,
            tc.tile_pool(name="psO", bufs=1, space="PSUM") as psO,
            tc.tile_pool(name="psSt", bufs=1, space="PSUM") as psSt,
        ):
            # ---- sum-of-squares chain, panel-pipelined (PE accumulates) ----
            for p in range(NPAN):
                cols = bass.ts(p, PANEL)
                ss_ps = psBig.tile([128, PANEL], F32, tag="big",
                                   name=f"ss_{p}")
                for c in range(NCHUNK):
                    x2c = x2p.tile([128, PANEL], BF16, tag=f"x2{c}",
                                   name=f"x2{c}_{p}")
                    if c == 0:
                        nc.scalar.square(x2c[:], xsb[:, c, cols])
                    elif c == 1:
                        nc.vector.tensor_mul(x2c[:], xsb[:, c, cols],
                                             xsb[:, c, cols])
                    else:
                        nc.gpsimd.tensor_mul(x2c[:], xsb[:, c, cols],
                                             xsb[:, c, cols])
                    nc.tensor.matmul(ss_ps[:], ones_sb[:], x2c[:],
                                     start=(c == 0), stop=(c == NCHUNK - 1))
                nc.scalar.copy(ssAll[:, cols], ss_ps[:])
                # per token-half: Ln then the two Exps (only ln/exp tables)
                if p % 2 == 1:
                    half = slice((p - 1) * PANEL, (p + 1) * PANEL)
                    nc.scalar.activation(lntAll[:, half], ssAll[:, half],
                                         AF.Ln)
                    nc.scalar.activation(sRep[:, half], lntAll[:, half],
                                         AF.Exp, bias=lsd_sb[:], scale=-0.5)
                    nc.scalar.activation(sRep2[:, half], lntAll[:, half],
                                         AF.Exp, scale=-1.0)

            def emit_proj_gate(p):
                cols = bass.ts(p, PANEL)
                gdst = [GK, GV, GQ, GZ]
                for g in range(NGRP):
                    gp = psBig.tile([128, PANEL], F32, tag="big",
                                    name=f"gp{g}_{p}")
                    for c in range(NCHUNK):
                        nc.tensor.matmul(gp[:], w_sb[:, c, g, :],
                                         xsb[:, c, cols],
                                         start=(c == 0), stop=(c == NCHUNK - 1))
                    # evict as plain copy (never blocks on sRep)
                    nc.scalar.copy(gdst[g][:, cols], gp[:])
                # s-scaling (waits on sRep/sRep2)
                if with_qkv_bias:
                    nc.vector.tensor_mul(GK[:, cols], GK[:, cols],
                                         sRep[:, cols])
                    nc.vector.tensor_scalar_add(GK[:, cols], GK[:, cols],
                                                bk_sb[:])
                    nc.vector.tensor_mul(GV[:, cols], GV[:, cols],
                                         sRep[:, cols])
                    nc.vector.tensor_scalar_add(GV[:, cols], GV[:, cols],
                                                bv_sb[:])
                    nc.vector.tensor_mul(GQ[:, cols], GQ[:, cols],
                                         sRep[:, cols])
                    nc.vector.tensor_scalar_add(GQ[:, cols], GQ[:, cols],
                                                bq_sb[:])
                else:
                    nc.vector.tensor_mul(GK[:, cols], GK[:, cols],
                                         sRep2[:, cols])
                nc.vector.tensor_mul(GZ[:, cols], GZ[:, cols], sRep[:, cols])

                # gate: araw = min(cumprod(1 + e^-(z+ba)), 3e37) = 1/a_cum
                u = gatp.tile([128, PANEL], F32, tag="u", name=f"u_{p}")
                nc.scalar.activation(u[:], GZ[:, cols], AF.Exp,
                                     bias=nba_sb[:], scale=-1.0)
                nc.vector.tensor_scalar_add(u[:], u[:], 1.0)
                init = 1.0 if p == 0 else araw[:, p * PANEL - 1:p * PANEL]
                nc.vector.tensor_tensor_scan(araw[:, cols], u[:], cap_sb[:],
                                             init, ALU.mult, ALU.min)
                nc.vector.tensor_scalar_min(ainvb[:, cols], araw[:, cols],
                                            1.0 / EPS_INV)
                acum = gatp.tile([128, PANEL], F32, tag="acum", name=f"ac_{p}")
                nc.vector.reciprocal_approx_fast(acum[:], araw[:, cols])
                nc.gpsimd.tensor_mul(GK[:, cols], GK[:, cols], ainvb[:, cols])
                nc.gpsimd.tensor_mul(GQ[:, cols], GQ[:, cols], acum[:])

            def emit_attention(p):
                for tl in range(PANEL // BLK):
                    t = 4 * p + tl
                    bc = bass.ts(t, BLK)
                    vkT = vktp.tile([128, 256], BF16, tag="vkt",
                                    name=f"vkt_{t}")
                    for h in range(2):
                        tr_ps = psTR.tile([128, 128], BF16, tag="tr",
                                          name=f"tr_{t}_{h}")
                        nc.tensor.transpose(tr_ps[:],
                                            (GK if h == 0 else GV)[:, bc],
                                            ident_sb[:])
                        if h == 0:
                            nc.vector.tensor_copy(vkT[:, 0:128], tr_ps[:])
                        else:
                            nc.scalar.copy(vkT[:, 128:256], tr_ps[:])
                    ssb = ssbp.tile([128, 256], BF16, tag="ssb",
                                    name=f"ssb_{t}")
                    for h in range(2):
                        hsl = slice(64 * h, 64 * (h + 1))
                        sp = psSP.tile([128, BLK], F32, tag="sp",
                                       name=f"sp_{t}_{h}")
                        nc.tensor.matmul(sp[:], GK[hsl, bc], GQ[hsl, bc],
                                         start=True, stop=True)
                        nc.vector.tensor_mul(ssb[:, 128 * h:128 * (h + 1)],
                                             sp[:], mask2_sb[:, 0:128])
                    o_ps = psO.tile([128, BLK], F32, tag="o", name=f"o_{t}")
                    for h in range(2):
                        hsl = slice(64 * h, 64 * (h + 1))
                        if t > 0:
                            nc.tensor.matmul(o_ps[hsl, :], stb[hsl, :],
                                             GQ[hsl, bc], start=True,
                                             stop=False)
                        nc.tensor.matmul(o_ps[hsl, :],
                                         vkT[:, 128 + 64 * h:192 + 64 * h],
                                         ssb[:, 128 * h:128 * (h + 1)],
                                         start=(t == 0), stop=True)
                    if t < NBLK - 1:
                        st_ps = psSt.tile([128, DH], F32, tag="st",
                                          name=f"st_{t}")
                        for h in range(2):
                            hsl = slice(64 * h, 64 * (h + 1))
                            nc.tensor.matmul(st_ps[hsl, :],
                                             vkT[:, 64 * h:64 * (h + 1)],
                                             vkT[:, 128 + 64 * h:192 + 64 * h],
                                             start=True, stop=True)
                        if t == 0:
                            nc.vector.tensor_copy(st[:], st_ps[:])
                        else:
                            nc.vector.tensor_add(st[:], st[:], st_ps[:])
                        nc.gpsimd.tensor_copy(stb[:], st[:])
                    osb = osbp.tile([128, BLK], BF16, tag="osb",
                                    name=f"osb_{t}")
                    if with_qkv_bias:
                        nc.scalar.copy(osb[:], o_ps[:])
                    else:
                        # apply the query-side s_i to output rows here
                        nc.vector.tensor_mul(osb[:], o_ps[:], sRep[:, bc])
                    y_ps = psBig.tile([128, DIM], F32, tag="big",
                                      name=f"y_{t}")
                    nc.tensor.matmul(y_ps[:], osb[:], wout_sb[:],
                                     start=True, stop=True)
                    ysb = ysbp.tile([128, DIM], BF16, tag="ysb",
                                    name=f"ysb_{t}")
                    nc.vector.tensor_copy(ysb[:, 0:DIM // 2],
                                          y_ps[:, 0:DIM // 2])
                    nc.scalar.copy(ysb[:, DIM // 2:], y_ps[:, DIM // 2:])
                    nc.sync.dma_start(y_d[t * BLK:(t + 1) * BLK, :], ysb[:])

            for p in range(NPAN):
                emit_proj_gate(p)
                if p >= SKEW:
                    emit_attention(p - SKEW)
            for p in range(NPAN - SKEW, NPAN):
                emit_attention(p)
    nc.finalize()
    return nc


_NC_CACHE = {}


def _get_nc(with_qkv_bias: bool):
    if with_qkv_bias not in _NC_CACHE:
        _NC_CACHE[with_qkv_bias] = build_nc(with_qkv_bias)
    return _NC_CACHE[with_qkv_bias]


def make_in_maps(x, gamma, w_qkv, b_qkv, w_a, b_a, w_out, b_out, with_qkv_bias):
    x = np.asarray(x, np.float32)
    gamma = np.asarray(gamma, np.float32)
    w_qkv = np.asarray(w_qkv, np.float32)
    b_qkv = np.asarray(b_qkv, np.float32)
    w_a = np.asarray(w_a, np.float32)
    b_a = np.asarray(b_a, np.float32)
    w_out = np.asarray(w_out, np.float32)

    wq = w_qkv[:, 0:DIM] * gamma[:, None] * SCALE
    wk = w_qkv[:, DIM:2 * DIM] * gamma[:, None]
    if not with_qkv_bias:
        wk = wk * float(DIM)  # with s^2 = DIM/ss folded: k'' = k * DIM/ss
    wv = w_qkv[:, 2 * DIM:3 * DIM] * gamma[:, None]
    wa = w_a * gamma[:, None]
    mask = np.triu(np.ones((128, 128), np.float32))  # [kt, qt] keep kt<=qt
    ident = np.eye(128, dtype=np.float32)

    xTs = []
    for b in range(B):
        xT = x[b].T.reshape(NCHUNK, 128, N).transpose(1, 0, 2)
        xTs.append(np.ascontiguousarray(xT.astype(ml_dtypes.bfloat16)))

    in_maps = []
    for core in range(HEADS):
        b, pair = divmod(core, 4)
        h0, h1 = 2 * pair, 2 * pair + 1
        s0 = slice(h0 * DH, (h0 + 1) * DH)
        s1 = slice(h1 * DH, (h1 + 1) * DH)
        groups = [
            np.concatenate([wk[:, s0], wk[:, s1]], axis=1),
            np.concatenate([wv[:, s0], wv[:, s1]], axis=1),
            np.concatenate([wq[:, s0], wq[:, s1]], axis=1),
            np.concatenate([wa[:, s0], wa[:, s1]], axis=1),
        ]
        w_all = np.stack(groups, axis=1).reshape(NCHUNK, 128, NGRP, 128)
        w_all = np.ascontiguousarray(
            w_all.transpose(1, 0, 2, 3).astype(ml_dtypes.bfloat16))
        m = {
            "xT": xTs[b],
            "wall": w_all,
            "wout": np.ascontiguousarray(
                np.concatenate([w_out[s0, :], w_out[s1, :]], axis=0)
                .astype(ml_dtypes.bfloat16)),
            "nba": np.ascontiguousarray(
                -np.concatenate([b_a[s0], b_a[s1]])[:, None].astype(np.float32)),
            "mask2": np.ascontiguousarray(
                np.concatenate([mask, mask], axis=1).astype(ml_dtypes.bfloat16)),
            "ident": np.ascontiguousarray(ident.astype(ml_dtypes.bfloat16)),
        }
        if with_qkv_bias:
            bq = b_qkv[0:DIM] * SCALE
            bk = b_qkv[DIM:2 * DIM]
            bv = b_qkv[2 * DIM:3 * DIM]
            m["bk"] = np.ascontiguousarray(
                np.concatenate([bk[s0], bk[s1]])[:, None].astype(np.float32))
            m["bv"] = np.ascontiguousarray(
                np.concatenate([bv[s0], bv[s1]])[:, None].astype(np.float32))
            m["bq"] = np.ascontiguousarray(
                np.concatenate([bq[s0], bq[s1]])[:, None].astype(np.float32))
        in_maps.append(m)
    return in_maps


def kernel(x, gamma, w_qkv, b_qkv, w_a, b_a, w_out, b_out, _profile=None):
    with_qkv_bias = bool(np.any(np.asarray(b_qkv)))
    nc = _get_nc(with_qkv_bias)
    in_maps = make_in_maps(x, gamma, w_qkv, b_qkv, w_a, b_a, w_out, b_out,
                           with_qkv_bias)
    kwargs = dict(_profile) if _profile else {}
    res = run_bass_kernel_spmd(nc, in_maps, core_ids=list(range(HEADS)),
                               **kwargs)
    if _profile is not None:
        _profile["result"] = res
    out = np.zeros((B, N, DIM), np.float32)
    for core in range(HEADS):
        out[core // 4] += res.results[core]["ypart"].astype(np.float32)
    out += np.asarray(b_out, np.float32)[None, None, :]
    return out
